# revision 1
# baseline (speedup 1.0000x reference)
"""AlphaFold3Loss Bass kernel for 8 TRN2 NeuronCores.

Sharding: distogram rows (768 -> 96/core), LDDT atom-rows (3072 -> 384/core),
MSE reductions replicated-free (each core does its own row range... core 0 only
actually -- tiny). Device computes all O(N^2) work; host combines scalar
partials (incl. 3x3 SVD for the Kabsch trace term).

Distogram math per pair (i,j):
  err = logsumexp_b(L_b) - L_tb,  tb = #(bounds < d2_ij)
  L_tb = L_0 + sum_b 1[bound_b < d2] * (L_{b+1} - L_b)   (telescoping)
Only the global sum of err is needed -> per-partition accumulators.

LDDT per pair: dp/dg from PE K=5 augmented matmuls (d2 = rn_m + rn_n - 2<x,x>
+ d_eps in one matmul), delta' = max(|dp-dg|, 30*(dg>=15)) so sigmoid terms of
cutoff-masked pairs vanish; sigmoid sums via ACT accum_out. Diagonal pairs are
included on device and subtracted on host.
"""
import sys
sys.path.insert(0, '/opt/trn_rl_repo')
import numpy as np
from contextlib import ExitStack

NT, NO_BINS, NA = 768, 64, 3072
NCORES = 8
RPC = NT // NCORES          # 96 distogram rows per core
APC = NA // NCORES          # 384 lddt atom-rows per core
NMT = RPC // 8              # 12 mega-tiles (8 rows each)
D_EPS = 1e-3                # lddt sqrt guard (host-corrected; see module doc)
BIG = 30.0

# cpak column layout
C_BOUNDS = 0                # 378 = 6*63
C_D2T = 378                 # 576 = 6*96  (d2t[p, k, i] = d2[row_i, 6p+k])
C_BIAS = 954                # 4 sigmoid biases
C_PRED = 958                # 72
C_PREDM = 1030              # 72
C_GT = 1102                 # 72
C_GTM = 1174                # 72
C_MW = 1246                 # 24
CW = 1280
# pgx column layout (partition dim = 5)
P_PW, P_GW, P_PX, P_GX, PGW = 0, 384, 768, 3840, 6912
# out column layout
O_S = 0                     # 576 per-pair sum-exp
O_B = 576                   # 12 ind*g sums
O_L0 = 588                  # 12 L0 sums
O_CC = 600                  # 18 c counts
O_SIG = 618                 # 12 sigmoid accums (3 chunks x 4 k)
O_MSE = 630                 # 18: Sw, A3(3), B3(3), M9(9), Spp, Sgg
OW = 672

_cache = {}


def _build_graph():
    from concourse import bass, bacc, tile, mybir
    F32 = mybir.dt.float32
    BF16 = mybir.dt.bfloat16
    U32 = mybir.dt.uint32
    AF = mybir.ActivationFunctionType
    ALU = mybir.AluOpType
    AX = mybir.AxisListType

    nc = bacc.Bacc(None, target_bir_lowering=False)
    lg_ext = nc.declare_dram_parameter("logits", [128, RPC, 384], F32, isOutput=False)
    cp_ext = nc.declare_dram_parameter("cpak", [128, CW], F32, isOutput=False)
    px_ext = nc.declare_dram_parameter("pgx", [5, PGW], F32, isOutput=False)
    out_ext = nc.declare_dram_parameter("out", [128, OW], F32, isOutput=True)

    with tile.TileContext(nc) as tc, ExitStack() as ctx:
        const = ctx.enter_context(tc.tile_pool(name="const", bufs=1))
        lpool = ctx.enter_context(tc.tile_pool(name="lp", bufs=2))
        epool = ctx.enter_context(tc.tile_pool(name="ep", bufs=1))
        wpool = ctx.enter_context(tc.tile_pool(name="wp", bufs=1))
        spool = ctx.enter_context(tc.tile_pool(name="sp", bufs=2))
        psum = ctx.enter_context(tc.tile_pool(name="ps", bufs=4, space="PSUM"))

        cpak = const.tile([128, CW], F32)
        nc.sync.dma_start(cpak[:], cp_ext[:, :])
        pgx = const.tile([5, PGW], F32)
        nc.sync.dma_start(pgx[:], px_ext[:, :])
        outb = const.tile([128, OW], F32)
        nc.vector.memset(outb[:], 0.0)
        dpr = const.tile([128, 18 * 512], BF16)   # stored |delta'| for phase B

        # ---------------- LDDT phase A: matmuls + sqrt + delta' ----------
        for rb in range(3):
            for cs in range(6):
                s = rb * 6 + cs
                psP = psum.tile([128, 512], F32, tag="psP")
                nc.tensor.matmul(psP[:], lhsT=pgx[:, P_PW + rb * 128:P_PW + rb * 128 + 128],
                                 rhs=pgx[:, P_PX + cs * 512:P_PX + (cs + 1) * 512],
                                 start=True, stop=True)
                psG = psum.tile([128, 512], F32, tag="psG")
                nc.tensor.matmul(psG[:], lhsT=pgx[:, P_GW + rb * 128:P_GW + rb * 128 + 128],
                                 rhs=pgx[:, P_GX + cs * 512:P_GX + (cs + 1) * 512],
                                 start=True, stop=True)
                dp = spool.tile([128, 512], F32, tag="dp")
                nc.scalar.activation(dp[:], psP[:], AF.Sqrt)
                dg = spool.tile([128, 512], F32, tag="dg")
                nc.scalar.activation(dg[:], psG[:], AF.Sqrt)
                delta = spool.tile([128, 512], F32, tag="delta")
                nc.vector.tensor_sub(delta[:], dp[:], dg[:])
                nc.vector.tensor_scalar(delta[:].bitcast(U32), delta[:].bitcast(U32),
                                        0x7FFFFFFF, None, ALU.bitwise_and)
                cbar = spool.tile([128, 512], F32, tag="cbar")
                nc.vector.tensor_scalar(cbar[:], dg[:], 15.0, BIG, ALU.is_ge, ALU.mult)
                nc.vector.tensor_tensor(dpr[:, s * 512:(s + 1) * 512], delta[:], cbar[:], ALU.max)
                cmask = spool.tile([128, 512], F32, tag="cmask")
                nc.vector.tensor_scalar(cmask[:], dg[:], 15.0, None, ALU.is_lt)
                nc.vector.tensor_reduce(outb[:, O_CC + s:O_CC + s + 1], cmask[:], AX.X, ALU.add)

        # ---------------- LDDT phase B: sigmoid sums ---------------------
        for ch in range(3):
            for k in range(4):
                sg = spool.tile([128, 3072], BF16, tag="sg")
                sac = spool.tile([128, 1], F32, tag="sac")
                nc.scalar.activation(sg[:], dpr[:, ch * 3072:(ch + 1) * 3072], AF.Sigmoid,
                                     bias=cpak[:, C_BIAS + k:C_BIAS + k + 1], scale=-1.0,
                                     accum_out=sac[:])
                nc.vector.tensor_copy(outb[:, O_SIG + ch * 4 + k:O_SIG + ch * 4 + k + 1], sac[:])

        # ---------------- MSE reductions (tiny) --------------------------
        def v3(col):
            return cpak[:, col:col + 72].rearrange("p (a c) -> p a c", c=3)
        pred, predm, gt, gtm = v3(C_PRED), v3(C_PREDM), v3(C_GT), v3(C_GTM)
        mw = cpak[:, C_MW:C_MW + 24]
        t24 = wpool.tile([128, 24], F32, tag="t24")
        nc.vector.tensor_reduce(outb[:, O_MSE:O_MSE + 1], mw, AX.X, ALU.add)  # Sw
        for i in range(3):  # A3 = sum mw*gt, B3 = sum mw*pred
            nc.vector.tensor_reduce(outb[:, O_MSE + 1 + i:O_MSE + 2 + i], gtm[:, :, i], AX.X, ALU.add)
            nc.vector.tensor_reduce(outb[:, O_MSE + 4 + i:O_MSE + 5 + i], predm[:, :, i], AX.X, ALU.add)
        for i in range(3):  # M9[i,j] = sum mw*pred_i*gt_j
            for j in range(3):
                nc.vector.tensor_mul(t24[:], predm[:, :, i], gt[:, :, j])
                c = O_MSE + 7 + 3 * i + j
                nc.vector.tensor_reduce(outb[:, c:c + 1], t24[:], AX.X, ALU.add)
        acc1 = wpool.tile([128, 3], F32, tag="acc1")
        for i in range(3):  # Spp partial per coord
            nc.vector.tensor_mul(t24[:], predm[:, :, i], pred[:, :, i])
            nc.vector.tensor_reduce(acc1[:, i:i + 1], t24[:], AX.X, ALU.add)
        nc.vector.tensor_reduce(outb[:, O_MSE + 16:O_MSE + 17], acc1[:], AX.X, ALU.add)
        for i in range(3):  # Sgg
            nc.vector.tensor_mul(t24[:], gtm[:, :, i], gt[:, :, i])
            nc.vector.tensor_reduce(acc1[:, i:i + 1], t24[:], AX.X, ALU.add)
        nc.vector.tensor_reduce(outb[:, O_MSE + 17:O_MSE + 18], acc1[:], AX.X, ALU.add)

        # ---------------- distogram ------------------------------------
        bounds4 = cpak[:, C_BOUNDS:C_BOUNDS + 378].rearrange(
            "p (k b) -> p k b", k=6).unsqueeze(1).broadcast_to([128, 8, 6, 63])
        d2t = cpak[:, C_D2T:C_D2T + 576].rearrange("p (k i) -> p k i", k=6)
        for mt in range(NMT):
            L = lpool.tile([128, 8 * 384], F32, tag="L")
            nc.sync.dma_start(L[:], lg_ext[:, 8 * mt:8 * mt + 8, :])
            L4 = L[:].rearrange("p (r k b) -> p r k b", r=8, k=6)
            E = epool.tile([128, 8 * 384], F32, tag="E")
            nc.scalar.activation(E[:], L[:], AF.Exp)
            nc.vector.tensor_reduce(outb[:, O_S + mt * 48:O_S + (mt + 1) * 48],
                                    E[:].rearrange("p (a b) -> p a b", b=64), AX.X, ALU.add)
            ind = wpool.tile([128, 8 * 378], F32, tag="ind")
            ind4 = ind[:].rearrange("p (r k b) -> p r k b", r=8, k=6)
            d2b = d2t[:, :, 8 * mt:8 * mt + 8].rearrange("p k i -> p i k") \
                .unsqueeze(3).broadcast_to([128, 8, 6, 63])
            nc.vector.tensor_tensor(ind4, bounds4, d2b, ALU.is_lt)
            g = wpool.tile([128, 8 * 378], F32, tag="g")
            g4 = g[:].rearrange("p (r k b) -> p r k b", r=8, k=6)
            nc.vector.tensor_tensor(g4, L4[:, :, :, 1:64], L4[:, :, :, 0:63], ALU.subtract)
            nc.vector.tensor_mul(g[:], ind[:], g[:])
            nc.vector.tensor_reduce(outb[:, O_B + mt:O_B + mt + 1], g[:], AX.X, ALU.add)
            nc.vector.tensor_reduce(outb[:, O_L0 + mt:O_L0 + mt + 1],
                                    L4[:, :, :, 0], AX.XY, ALU.add)

        nc.sync.dma_start(out_ext[:, :], outb[:])
    nc.finalize()
    return nc


def _host_prep(inputs):
    lg = np.ascontiguousarray(inputs["distogram_logits"][0], dtype=np.float32)  # [768,768,64]
    pos = np.asarray(inputs["all_atom_positions"][0], dtype=np.float32)
    pred = np.asarray(inputs["denoised_atoms"][0], dtype=np.float32)            # [3072,3]
    gt = np.asarray(inputs["augmented_gt_atoms"][0], dtype=np.float32)
    ae = np.asarray(inputs["atom_exists"][0], dtype=np.float32)

    pb = pos[:, 1, :]                                        # CA positions [768,3]
    diff = pb[:, None, :] - pb[None, :, :]
    d2 = np.einsum('ijk,ijk->ij', diff, diff).astype(np.float32)   # [768,768]

    bounds63 = (np.linspace(0.0, 32.0, 63) ** 2).astype(np.float32)
    bounds378 = np.tile(bounds63, 6)

    def augW(x):  # stationary form [5, n]
        rn = (x.astype(np.float64) ** 2).sum(-1)
        return np.stack([-2 * x[:, 0], -2 * x[:, 1], -2 * x[:, 2],
                         (rn + D_EPS).astype(np.float32), np.ones(len(x), np.float32)]).astype(np.float32)

    def augX(x):  # moving form [5, n]
        rn = (x.astype(np.float64) ** 2).sum(-1)
        return np.stack([x[:, 0], x[:, 1], x[:, 2],
                         np.ones(len(x), np.float32), rn.astype(np.float32)]).astype(np.float32)

    pW, gW, pX, gX = augW(pred), augW(gt), augX(pred), augX(gt)
    mw = (ae * ae).astype(np.float32)
    predm = (pred * mw[:, None]).astype(np.float32)
    gtm = (gt * mw[:, None]).astype(np.float32)

    in_maps = []
    for c in range(NCORES):
        rows = slice(RPC * c, RPC * (c + 1))
        lgc = lg[rows].reshape(RPC, 128, 384).transpose(1, 0, 2)
        lgc = np.ascontiguousarray(lgc)
        cpak = np.zeros((128, CW), np.float32)
        cpak[:, C_BOUNDS:C_BOUNDS + 378] = bounds378
        # d2t[p, k, i] = d2[row_i, 6p+k]
        d2c = d2[rows].T.reshape(128, 6, RPC)
        cpak[:, C_D2T:C_D2T + 576] = d2c.reshape(128, 576)
        cpak[:, C_BIAS:C_BIAS + 4] = np.array([0.5, 1.0, 2.0, 4.0], np.float32)
        cpak[:, C_PRED:C_PRED + 72] = pred.reshape(128, 72)
        cpak[:, C_PREDM:C_PREDM + 72] = predm.reshape(128, 72)
        cpak[:, C_GT:C_GT + 72] = gt.reshape(128, 72)
        cpak[:, C_GTM:C_GTM + 72] = gtm.reshape(128, 72)
        cpak[:, C_MW:C_MW + 24] = mw.reshape(128, 24)
        pgx = np.zeros((5, PGW), np.float32)
        arows = slice(APC * c, APC * (c + 1))
        pgx[:, P_PW:P_PW + 384] = pW[:, arows]
        pgx[:, P_GW:P_GW + 384] = gW[:, arows]
        pgx[:, P_PX:P_PX + NA] = pX
        pgx[:, P_GX:P_GX + NA] = gX
        in_maps.append({"logits": lgc, "cpak": cpak, "pgx": pgx})
    return in_maps


def _host_combine(outs, inputs):
    tm = np.asarray(inputs["token_mask"][0], dtype=np.float64)
    ae = np.asarray(inputs["atom_exists"][0], dtype=np.float64)
    ts = float(np.asarray(inputs["timesteps"])[0, 0])

    errsum = 0.0
    num_sig = 0.0
    den_c = 0.0
    for o in outs:
        o = o.astype(np.float64)
        S = o[:, O_S:O_S + 576]
        errsum += np.log(S).sum()
        errsum -= o[:, O_B:O_B + 12].sum() + o[:, O_L0:O_L0 + 12].sum()
        num_sig += o[:, O_SIG:O_SIG + 12].sum()
        den_c += o[:, O_CC:O_CC + 18].sum()

    denom = 1e-6 + tm.sum() ** 2
    l_disto = errsum / denom

    # diagonal removal: each atom contributes c=1 and sigmas at delta~0
    sig0 = sum(1.0 / (1.0 + np.exp(-(k))) for k in (0.5, 1.0, 2.0, 4.0))
    num = num_sig / 4.0 - NA * sig0 / 4.0
    den = den_c - NA
    lddt = num / (den + 1e-5)
    l_lddt = 1.0 - lddt

    # MSE from core 0's reductions
    m = outs[0][:, O_MSE:O_MSE + 18].astype(np.float64).sum(0)
    Sw, A3, B3 = m[0], m[1:4], m[4:7]
    M9 = m[7:16].reshape(3, 3)
    Spp, Sgg = m[16], m[17]
    wsum = Sw + 1e-5
    mu = A3 / wsum          # gt centroid
    mugt = B3 / wsum        # pred centroid
    H = M9 - np.outer(mugt, A3) - np.outer(B3, mu) + Sw * np.outer(mugt, mu)
    U, s, Vt = np.linalg.svd(H)
    d = np.sign(np.linalg.det(U @ Vt))
    tr = s[0] + s[1] + d * s[2]
    Swg = Spp - 2 * B3 @ mugt + Sw * (mugt @ mugt)
    Swc = Sgg - 2 * A3 @ mu + Sw * (mu @ mu)
    msesum = Swg + Swc - 2 * tr + 1e-5 * Sw
    mse = msesum / (1e-5 + ae.sum()) / 3.0
    scale = (ts ** 2 + 16.0 ** 2) / ((ts * 16.0) ** 2 + 1e-5)
    l_mse = scale * mse

    total = 0.03 * l_disto + 1.0 * l_lddt + 4.0 * l_mse
    return np.float32(total)


def _run(inputs, trace=False):
    from concourse.bass_utils import run_bass_kernel_spmd
    if "nc" not in _cache:
        _cache["nc"] = _build_graph()
    nc = _cache["nc"]
    in_maps = _host_prep(inputs)
    res = run_bass_kernel_spmd(nc, in_maps, list(range(NCORES)), trace=trace)
    outs = [res.results[c]["out"] for c in range(NCORES)]
    return _host_combine(outs, inputs), res


def kernel(**inputs):
    out, _ = _run(inputs, trace=False)
    return out


def kernel_traced(**inputs):
    return _run(inputs, trace=True)
